# revision 2
# baseline (speedup 1.0000x reference)
"""CrystalTransformer (TransformerConv x3 + segment-mean pool) on 8 trn2 cores.

Low-instruction-count redesign: the statically-unrolled per-tile pipeline of the
previous version (~19k instructions/core) dominated wall time through bass
tracing, walrus compile, and NEFF load. This version keeps the same math but
emits ~2k instructions/core:

- Per layer, a static "gather phase" stages h[src] for every 128-edge tile into
  a DRAM X_stage laid out [block][128 rows][tile][116 cols] whose edge-feature
  columns ([ea(50) | 1 | drel]) are prefilled once via one DRAM->DRAM DMA.
- All compute runs in tc.For_i hardware loops over the 20 dst blocks (indirect
  DMAs cannot live inside For_i bodies, everything else can).
- The per-edge q[dst] gather is replaced by a one-hot matmul: qd = S^T^T @ Q_b
  using the transposed scatter one-hot, so q never round-trips DRAM.
- Softmax denominator rides along as a strided exm copy into the scatter
  payload (col 64 of each head's 65-wide slot).
- Padded edge slots carry drel=-1 which zeroes their one-hot row: no masks.
"""
import numpy as np

P = 128
N, E, G = 20000, 320000, 256
DA, DE, D, H, L = 92, 50, 64, 4, 3
NCORES = 8
NLOC = 2560            # node slots per core (20 blocks of 128)
NB = NLOC // P         # 20 dst blocks per core
NPAD = NLOC * NCORES   # 20480
XW = D + DE + 2        # 116 = [h_src(64) | ea(50) | 1 | drel]


# ---------------------------------------------------------------- BIR patch --
def _install_birpatch():
    """This container's walrus rejects >1 sem wait per instruction; hoist
    extras onto injected preceding Drains (same engine => same order)."""
    import json
    import concourse.bass2jax as b2j
    if getattr(b2j, "_birpatch_installed", False):
        return
    orig = b2j.compile_bir_kernel

    def patch(bir_bytes):
        d = json.loads(bir_bytes)
        for fn in d.get("functions", []):
            for blk in fn.get("blocks", []):
                out = []
                for ins in blk.get("instructions", []):
                    si = ins.get("sync_info") or {}
                    waits = si.get("on_wait") or []
                    if len(waits) > 1:
                        for k, w in enumerate(waits[:-1]):
                            out.append({
                                "debug": ins.get("debug", 0),
                                "engine": ins["engine"], "ins": [], "outs": [],
                                "name": f'{ins["name"]}-w{k}', "opcode": "Drain",
                                "sync_info": {"on_update": [], "on_wait": [w]},
                            })
                        si["on_wait"] = waits[-1:]
                    out.append(ins)
                blk["instructions"] = out
        return json.dumps(d).encode()

    def wrapper(bir_str, *a, **kw):
        try:
            bir_str = patch(bir_str)
        except Exception as e:  # pragma: no cover
            print("[birpatch] failed:", e)
        return orig(bir_str, *a, **kw)

    b2j.compile_bir_kernel = wrapper
    b2j._birpatch_installed = True


# ------------------------------------------------------------------- device --
def _build_nc(tpb):
    import contextlib
    import concourse.bass as bass
    import concourse.mybir as mybir
    import concourse.tile as tile
    from concourse.bass import ds, ts
    from concourse.masks import make_identity

    f32, i32 = mybir.dt.float32, mybir.dt.int32
    Alu, Act = mybir.AluOpType, mybir.ActivationFunctionType

    nc = bass.Bass("TRN2", target_bir_lowering=False, debug=False,
                   num_devices=NCORES)
    di = lambda nm, sh, dt=f32: nc.dram_tensor(nm, sh, dt, kind="ExternalInput")
    x_in = di("x_shard", [NLOC, DA])
    ea_in = di("ea_pack", [P, NB, tpb, DE + 2])
    idx_in = di("idx_pack", [P, NB * tpb], i32)
    brel_in = di("brel_pack", [P, NB])
    watom_in = di("w_atom_aug", [DA + 1, D])
    wkv_in = di("wkv", [L, XW, 2 * H * D])
    wqs_in = di("wqs", [L, D + 1, H * D + D])
    out_pool = nc.dram_tensor("out_pool", [P, D + 1], f32, kind="ExternalOutput")
    import os
    dbg = os.environ.get("K2_DEBUG_H") == "1"
    h_dbg = nc.dram_tensor("h_dbg", [L + 1, NLOC, D], f32,
                           kind="ExternalOutput") if dbg else None

    h_mine = nc.dram_tensor("h_mine", [NLOC, D], f32)
    h_full = [nc.dram_tensor(f"h_full_{l}", [NPAD, D], f32, addr_space="Shared")
              for l in range(L)]
    x_stage = nc.dram_tensor("x_stage", [P, NB, tpb, XW], f32)

    HD = H * D

    with tile.TileContext(nc, num_cores=NCORES) as tc:
        with contextlib.ExitStack() as st:
            cp = st.enter_context(tc.tile_pool(name="const", bufs=1))
            io = st.enter_context(tc.tile_pool(name="io", bufs=3))
            vp = st.enter_context(tc.tile_pool(name="dve", bufs=3))
            ps_t = st.enter_context(tc.tile_pool(name="ps_t", bufs=2, space="PSUM"))
            ps_k = st.enter_context(tc.tile_pool(name="ps_k", bufs=2, space="PSUM"))
            ps_z = st.enter_context(tc.tile_pool(name="ps_z", bufs=1, space="PSUM"))
            ps_b = st.enter_context(tc.tile_pool(name="ps_b", bufs=1, space="PSUM"))
            ps_q = st.enter_context(tc.tile_pool(name="ps_q", bufs=1, space="PSUM"))
            ps_p = st.enter_context(tc.tile_pool(name="ps_p", bufs=1, space="PSUM"))

            ident = cp.tile([P, P], f32)
            make_identity(nc, ident[:])
            iota_i = cp.tile([P, P], i32)
            nc.gpsimd.iota(iota_i[:], pattern=[[1, P]], base=0, channel_multiplier=0)
            iota_f = cp.tile([P, P], f32)
            nc.vector.tensor_copy(iota_f[:], iota_i[:])
            ones_col = cp.tile([P, 1], f32)
            nc.vector.memset(ones_col[:], 1.0)
            h_loc = cp.tile([P, NB * D], f32)
            skip_loc = cp.tile([P, NB * D], f32)
            q_loc = cp.tile([P, NB * HD], f32)
            watom_sb = cp.tile([DA + 1, D], f32)
            nc.sync.dma_start(out=watom_sb[:], in_=watom_in[:])
            idx_all = cp.tile([P, NB * tpb], i32)
            nc.sync.dma_start(out=idx_all[:], in_=idx_in[:])
            brel_all = cp.tile([P, NB], f32)
            nc.sync.dma_start(out=brel_all[:], in_=brel_in[:])

            # prefill edge-feature columns of X_stage (cols 64:116), once
            nc.sync.dma_start(out=x_stage[:, :, :, D:], in_=ea_in[:])

            # ---- embed: h0 = x@W_atom + b_atom
            with tc.For_i(0, NB, 1) as b:
                xb = io.tile([P, DA], f32, tag="xb")
                nc.sync.dma_start(out=xb[:], in_=x_in[ds(b * P, P), :])
                xT_ps = ps_t.tile([P, P], f32, tag="tr")
                nc.tensor.transpose(out=xT_ps[:DA, :], in_=xb[:], identity=ident[:])
                xT = io.tile([DA + 1, P], f32, tag="xT")
                nc.vector.memset(xT[:], 1.0)
                nc.vector.tensor_copy(xT[:DA, :], xT_ps[:DA, :])
                hb_ps = ps_b.tile([P, HD + D], f32, tag="blk")
                nc.tensor.matmul(hb_ps[:, :D], lhsT=xT[:], rhs=watom_sb[:],
                                 start=True, stop=True)
                nc.vector.tensor_copy(h_loc[:, ts(b, D)], hb_ps[:, :D])
                hstage = io.tile([P, D], f32, tag="hstage")
                nc.vector.tensor_copy(hstage[:], hb_ps[:, :D])
                nc.sync.dma_start(out=h_mine[ds(b * P, P), :], in_=hstage[:])
            if dbg:
                for b in range(NB):
                    nc.sync.dma_start(out=h_dbg[0, b * P:(b + 1) * P, :],
                                      in_=h_loc[:, b * D:(b + 1) * D])
            tc.strict_bb_all_engine_barrier()
            nc.gpsimd.collective_compute(
                "AllGather", Alu.bypass,
                replica_groups=[list(range(NCORES))],
                ins=[h_mine.ap().opt()], outs=[h_full[0].ap().opt()])
            tc.strict_bb_all_engine_barrier()

            for l in range(L):
                wkv_sb = cp.tile([XW, 2 * HD], f32, tag="wkv")
                nc.sync.dma_start(out=wkv_sb[:], in_=wkv_in[l])
                wqs_sb = cp.tile([D + 1, HD + D], f32, tag="wqs")
                nc.sync.dma_start(out=wqs_sb[:], in_=wqs_in[l])

                # ---- B1: q & skip per block (into SBUF q_loc/skip_loc)
                with tc.For_i(0, NB, 1) as b:
                    h_b = io.tile([P, D], f32, tag="h_b")
                    nc.vector.tensor_copy(h_b[:], h_loc[:, ts(b, D)])
                    hT_ps = ps_t.tile([P, P], f32, tag="tr")
                    nc.tensor.transpose(out=hT_ps[:D, :], in_=h_b[:], identity=ident[:])
                    hT = io.tile([D + 1, P], f32, tag="hT")
                    nc.vector.memset(hT[:], 1.0)
                    nc.vector.tensor_copy(hT[:D, :], hT_ps[:D, :])
                    qs_ps = ps_b.tile([P, HD + D], f32, tag="blk")
                    nc.tensor.matmul(qs_ps[:], lhsT=hT[:], rhs=wqs_sb[:],
                                     start=True, stop=True)
                    nc.scalar.copy(q_loc[:, ts(b, HD)], qs_ps[:, :HD])
                    nc.scalar.copy(skip_loc[:, ts(b, D)], qs_ps[:, HD:])

                # ---- G: stage h[src] for every tile (static; indirect DMA
                # cannot live inside For_i)
                for b in range(NB):
                    xg = io.tile([P, tpb * D], f32, tag="xg")
                    for t in range(tpb):
                        nc.gpsimd.indirect_dma_start(
                            out=xg[:, t * D:(t + 1) * D], out_offset=None,
                            in_=h_full[l][:],
                            in_offset=bass.IndirectOffsetOnAxis(
                                ap=idx_all[:, b * tpb + t:b * tpb + t + 1], axis=0))
                    nc.sync.dma_start(
                        out=x_stage[:, b, :, :D],
                        in_=xg[:].rearrange("p (t c) -> p t c", c=D))
                tc.strict_bb_all_engine_barrier()

                # ---- B2: edge tiles + combine, one For_i over blocks
                with tc.For_i(0, NB, 1) as b:
                    X_all = io.tile([P, tpb * XW], f32, tag="X_all")
                    nc.sync.dma_start(
                        out=X_all[:],
                        in_=x_stage[:, ds(b, 1), :, :].rearrange("p o t c -> p (o t c)"))
                    q_b = io.tile([P, HD], f32, tag="q_b")
                    nc.vector.tensor_copy(q_b[:], q_loc[:, ts(b, HD)])
                    z_ps = ps_z.tile([P, H * (D + 1)], f32, tag="z")
                    for t in range(tpb):
                        Xs = X_all[:, t * XW:(t + 1) * XW]
                        XT_ps = ps_t.tile([P, P], f32, tag="tr")
                        nc.tensor.transpose(out=XT_ps[:XW, :], in_=Xs,
                                            identity=ident[:])
                        XT = io.tile([XW, P], f32, tag="XT")
                        nc.vector.tensor_copy(XT[:], XT_ps[:XW, :])
                        kv_ps = ps_k.tile([P, 2 * HD], f32, tag="kv")
                        nc.tensor.matmul(kv_ps[:], lhsT=XT[:], rhs=wkv_sb[:],
                                         start=True, stop=True)
                        S = vp.tile([P, P], f32, tag="S")
                        nc.gpsimd.tensor_scalar(
                            out=S[:], in0=iota_f[:],
                            scalar1=X_all[:, t * XW + XW - 1:(t + 1) * XW],
                            scalar2=None, op0=Alu.is_equal)
                        ST_ps = ps_t.tile([P, P], f32, tag="tr")
                        nc.tensor.transpose(out=ST_ps[:], in_=S[:], identity=ident[:])
                        ST = io.tile([P, P], f32, tag="ST")
                        nc.scalar.copy(ST[:], ST_ps[:])
                        qd_ps = ps_q.tile([P, HD], f32, tag="qd")
                        nc.tensor.matmul(qd_ps[:], lhsT=ST[:], rhs=q_b[:],
                                         start=True, stop=True)
                        qd_sb = vp.tile([P, HD], f32, tag="qd_sb")
                        nc.vector.tensor_copy(qd_sb[:], qd_ps[:])
                        prod = vp.tile([P, HD], f32, tag="prod")
                        nc.vector.tensor_tensor(out=prod[:], in0=kv_ps[:, :HD],
                                                in1=qd_sb[:], op=Alu.mult)
                        alpha = vp.tile([P, H], f32, tag="alpha")
                        nc.vector.tensor_reduce(
                            out=alpha[:],
                            in_=prod[:].rearrange("p (h d) -> p h d", d=D),
                            axis=mybir.AxisListType.X, op=Alu.add)
                        exm = vp.tile([P, H], f32, tag="exm")
                        nc.scalar.activation(exm[:], alpha[:], Act.Exp,
                                             scale=float(1.0 / np.sqrt(D)))
                        Vex = vp.tile([P, H * (D + 1)], f32, tag="Vex")
                        for h in range(H):
                            nc.vector.tensor_scalar_mul(
                                out=Vex[:, h * (D + 1):h * (D + 1) + D],
                                in0=kv_ps[:, HD + h * D:HD + (h + 1) * D],
                                scalar1=exm[:, h:h + 1])
                        nc.vector.tensor_copy(
                            Vex[:].rearrange("p (h c) -> p h c", c=D + 1)[:, :, D:],
                            exm[:])
                        nc.tensor.matmul(z_ps[:], lhsT=S[:], rhs=Vex[:],
                                         start=(t == 0), stop=(t == tpb - 1))

                    # ---- B3: combine per block
                    zv = z_ps[:].rearrange("p (h c) -> p h c", c=D + 1)
                    den = vp.tile([P, H], f32, tag="den")
                    nc.vector.tensor_scalar_max(out=den[:], in0=zv[:, :, D:],
                                                scalar1=1e-30)
                    rden = vp.tile([P, H], f32, tag="rden")
                    nc.vector.reciprocal(rden[:], den[:])
                    m = vp.tile([P, D], f32, tag="m")
                    tt = vp.tile([P, D], f32, tag="tt")
                    nc.vector.tensor_scalar_mul(out=m[:], in0=z_ps[:, 0:D],
                                                scalar1=rden[:, 0:1])
                    for h in range(1, H):
                        nc.vector.tensor_scalar_mul(
                            out=tt[:], in0=z_ps[:, h * (D + 1):h * (D + 1) + D],
                            scalar1=rden[:, h:h + 1])
                        nc.vector.tensor_tensor(out=m[:], in0=m[:], in1=tt[:],
                                                op=Alu.add)
                    sk = vp.tile([P, D], f32, tag="sk")
                    nc.scalar.copy(sk[:], skip_loc[:, ts(b, D)])
                    nc.vector.tensor_tensor(out=m[:], in0=m[:], in1=sk[:],
                                            op=Alu.add)
                    hstage = io.tile([P, D], f32, tag="hstage")
                    nc.vector.tensor_scalar_max(out=hstage[:], in0=m[:], scalar1=0.0)
                    nc.gpsimd.tensor_copy(h_loc[:, ts(b, D)], hstage[:])
                    if l < L - 1:
                        nc.sync.dma_start(out=h_mine[ds(b * P, P), :], in_=hstage[:])
                if dbg:
                    for b in range(NB):
                        nc.sync.dma_start(out=h_dbg[l + 1, b * P:(b + 1) * P, :],
                                          in_=h_loc[:, b * D:(b + 1) * D])
                if l < L - 1:
                    tc.strict_bb_all_engine_barrier()
                    nc.gpsimd.collective_compute(
                        "AllGather", Alu.bypass,
                        replica_groups=[list(range(NCORES))],
                        ins=[h_mine.ap().opt()], outs=[h_full[l + 1].ap().opt()])
                    tc.strict_bb_all_engine_barrier()

            # ---- pooling: one-hot on batch ids, PSUM accumulated over blocks
            pool_ps = ps_p.tile([P, D + 1], f32, tag="pool")

            for b in range(NB):
                Sb = vp.tile([P, P], f32, tag="Sb")
                nc.gpsimd.tensor_scalar(out=Sb[:], in0=iota_f[:],
                                        scalar1=brel_all[:, b:b + 1],
                                        scalar2=None, op0=Alu.is_equal)
                hp = io.tile([P, D + 1], f32, tag="hp")
                nc.vector.tensor_copy(hp[:, :D], h_loc[:, b * D:(b + 1) * D])
                nc.vector.memset(hp[:, D:], 1.0)
                nc.tensor.matmul(pool_ps[:], lhsT=Sb[:], rhs=hp[:],
                                 start=(b == 0), stop=(b == NB - 1),
                                 skip_group_check=True)

            pool_sb = vp.tile([P, D + 1], f32, tag="pool_sb")
            nc.vector.tensor_copy(pool_sb[:], pool_ps[:])
            nc.sync.dma_start(out=out_pool[:], in_=pool_sb[:])
    return nc


# --------------------------------------------------------------------- host --
def kernel(**inputs):
    _install_birpatch()
    from concourse.bass_utils import run_bass_kernel_spmd

    x = np.asarray(inputs["x"], np.float32)
    ei = np.asarray(inputs["edge_index"]).astype(np.int64)
    ea = np.asarray(inputs["edge_attr"], np.float32)
    batch = np.asarray(inputs["batch"]).astype(np.int64)
    Wq = np.asarray(inputs["Wq"], np.float32); bq = np.asarray(inputs["bq"], np.float32)
    Wk = np.asarray(inputs["Wk"], np.float32); bk = np.asarray(inputs["bk"], np.float32)
    Wv = np.asarray(inputs["Wv"], np.float32); bv = np.asarray(inputs["bv"], np.float32)
    We = np.asarray(inputs["We"], np.float32)
    Wskip = np.asarray(inputs["Wskip"], np.float32)
    bskip = np.asarray(inputs["bskip"], np.float32)
    W_atom = np.asarray(inputs["W_atom"], np.float32)
    b_atom = np.asarray(inputs["b_atom"], np.float32)
    W_edge = np.asarray(inputs["W_edge"], np.float32)
    b_edge = np.asarray(inputs["b_edge"], np.float32)
    W_out = np.asarray(inputs["W_out"], np.float32)
    b_out = np.asarray(inputs["b_out"], np.float32)

    HD = H * D
    src, dst = ei[0], ei[1]
    order = np.argsort(dst, kind="stable")
    src_s, dst_s = src[order], dst[order]
    ea_s = ea[order]

    nblk = NCORES * NB
    blk_of = dst_s // P
    counts = np.bincount(blk_of, minlength=nblk)
    starts = np.zeros(nblk + 1, np.int64)
    np.cumsum(counts, out=starts[1:])
    tpb = int(np.ceil(max(1, counts.max()) / P))

    # slot assignment, vectorized over all edges
    rank = np.arange(len(dst_s)) - starts[blk_of]
    t_of = (rank // P).astype(np.int64)
    r_of = (rank % P).astype(np.int64)
    core_of = blk_of // NB
    b_of = blk_of % NB

    idx_pack = np.zeros((NCORES, P, NB * tpb), np.int32)
    ea_pack = np.zeros((NCORES, P, NB, tpb, DE + 2), np.float32)
    ea_pack[:, :, :, :, DE + 1] = -1.0
    idx_pack[core_of, r_of, b_of * tpb + t_of] = src_s.astype(np.int32)
    ea_pack[core_of, r_of, b_of, t_of, :DE] = ea_s
    ea_pack[core_of, r_of, b_of, t_of, DE] = 1.0
    ea_pack[core_of, r_of, b_of, t_of, DE + 1] = (dst_s % P).astype(np.float32)

    # weight folds
    Wea = np.concatenate([W_edge, b_edge[None, :]], 0)        # [51, 64]
    wkv = np.zeros((L, XW, 2 * HD), np.float32)
    wqs = np.zeros((L, D + 1, HD + D), np.float32)
    for l in range(L):
        ew = Wea @ We[l]                                      # [51, 256]
        wkv[l, :D, :HD] = Wk[l]
        wkv[l, D:D + DE, :HD] = ew[:DE]
        wkv[l, D + DE, :HD] = ew[DE] + bk[l]
        wkv[l, :D, HD:] = Wv[l] / H
        wkv[l, D:D + DE, HD:] = ew[:DE] / H
        wkv[l, D + DE, HD:] = (ew[DE] + bv[l]) / H
        wqs[l, :D, :HD] = Wq[l]
        wqs[l, D, :HD] = bq[l]
        wqs[l, :D, HD:] = Wskip[l]
        wqs[l, D, HD:] = bskip[l]
    watom = np.concatenate([W_atom, b_atom[None, :]], 0)

    in_maps, g0s = [], []
    for c in range(NCORES):
        n0 = c * NLOC
        xs = np.zeros((NLOC, DA), np.float32)
        real = min(NLOC, max(0, N - n0))
        xs[:real] = x[n0:n0 + real]
        brel = np.full((P, NB), -1.0, np.float32)
        g0 = int(batch[min(n0, N - 1)]) if n0 < N else 0
        if real > 0:
            br = np.full(NLOC, -1.0, np.float32)
            br[:real] = (batch[n0:n0 + real] - g0).astype(np.float32)
            brel[:, :] = br.reshape(NB, P).T
        g0s.append(g0)
        in_maps.append({
            "x_shard": xs,
            "ea_pack": ea_pack[c],
            "idx_pack": idx_pack[c],
            "brel_pack": brel,
            "w_atom_aug": watom,
            "wkv": wkv,
            "wqs": wqs,
        })

    nc = _build_nc(tpb)
    res = run_bass_kernel_spmd(nc, in_maps, core_ids=list(range(NCORES)))

    sums = np.zeros((G + P, D), np.float64)
    cnts = np.zeros(G + P, np.float64)
    for c in range(NCORES):
        op = res.results[c]["out_pool"]
        sums[g0s[c]:g0s[c] + P] += op[:, :D]
        cnts[g0s[c]:g0s[c] + P] += op[:, D]
    pooled = sums[:G] / np.maximum(cnts[:G], 1.0)[:, None]
    out = pooled.astype(np.float32) @ W_out + b_out
    return out.squeeze()
